# revision 8
# baseline (speedup 1.0000x reference)
"""Trainium2 Bass kernel for nn_Attention_57672820850902.

Channel-attention block (XCA-style):
  kv = dwconv3x3(conv1x1(x)); k, v = split(kv)
  q  = conv3x3_full(conv1x1(y))
  q, k l2-normalized per channel row; attn = softmax(q @ k^T * temp) per head
  out = x - conv1x1(attn @ v)

Sharding: 8 cores = 4 batches x 2 spatial halves (64 rows + 1-row halo).
All convs are local to a core; the only cross-core data is a 26KB
pairwise AllReduce carrying per-channel sum-of-squares (for the l2 norm)
and the per-head 32x32 attention logits (contracted over local spatial).

Channel tiling: 192 channels = 128 ("a") + 64 ("b"). The v-path "b" half
lives in SBUF partitions 64..127 (tiles named *u) because the kv1 matmul
emits it there; compute engines cannot move data across partitions, and
matmul row-tiling handles base_partition=64 operands natively.
"""

import os
import numpy as np
import ml_dtypes

B, C, H, W, HEADS = 4, 192, 128, 128, 6
HC = C // HEADS                      # 32 channels per head
HP = H // 2                          # 64 rows per core
PH, PW = HP + 2, W + 2               # 66 x 130 padded shard
S_PAD = PH * PW                      # 8580
S_IN = HP * W                        # 8192
NCORES = 8
CA, CB = 128, 64                     # channel tile split of 192

bf16 = ml_dtypes.bfloat16

_cache = {}


def _pad_chunks():
    bounds = list(range(0, S_PAD, 512)) + [S_PAD]
    return list(zip(bounds[:-1], bounds[1:]))


def _build():
    import concourse.bass as bass
    import concourse.mybir as mybir
    import concourse.tile as tile
    from concourse import bacc

    dt = mybir.dt
    Alu = mybir.AluOpType
    Act = mybir.ActivationFunctionType

    nc = bacc.Bacc("TRN2", target_bir_lowering=False, debug=False,
                   num_devices=NCORES)

    # ---- per-core inputs ----
    x_pad_t = nc.dram_tensor("x_pad", [C, PH, PW], dt.bfloat16, kind="ExternalInput")
    y_pad_t = nc.dram_tensor("y_pad", [C, PH, PW], dt.bfloat16, kind="ExternalInput")
    x_ctr_t = nc.dram_tensor("x_ctr", [C, S_IN], dt.float32, kind="ExternalInput")
    # ---- weights (same on all cores) ----
    # kv_wT columns host-permuted to [k 0:128 | v 128:192 ; k 128:192 | v 0:128]
    kvw_t = nc.dram_tensor("kv_wT", [C, 2 * C], dt.bfloat16, kind="ExternalInput")
    qw_t = nc.dram_tensor("q_wT", [C, C], dt.bfloat16, kind="ExternalInput")
    qdw_t = nc.dram_tensor("qdw_T", [9, C, C], dt.bfloat16, kind="ExternalInput")
    dwk_t = nc.dram_tensor("dw_k", [C, 9], dt.float32, kind="ExternalInput")
    dwv_t = nc.dram_tensor("dw_v", [C, 9], dt.float32, kind="ExternalInput")
    projw_t = nc.dram_tensor("proj_wT", [C, C], dt.bfloat16, kind="ExternalInput")
    temp_t = nc.dram_tensor("temp", [HC, HEADS], dt.float32, kind="ExternalInput")
    out_t = nc.dram_tensor("out", [C, S_IN], dt.float32, kind="ExternalOutput")

    PCH = _pad_chunks()
    taps = [(ky, kx) for ky in range(3) for kx in range(3)]

    with tile.TileContext(nc) as tc:
        with tc.tile_pool(name="w", bufs=1) as wp, \
             tc.tile_pool(name="big", bufs=1) as bigp, \
             tc.tile_pool(name="io", bufs=3) as iop, \
             tc.tile_pool(name="io2", bufs=2) as iop2, \
             tc.tile_pool(name="tp", bufs=4) as tpp, \
             tc.tile_pool(name="ev", bufs=4) as evp, \
             tc.tile_pool(name="ev2", bufs=2) as evp2, \
             tc.tile_pool(name="small", bufs=1) as smp, \
             tc.tile_pool(name="ps", bufs=6, space="PSUM") as psp, \
             tc.tile_pool(name="psattn", bufs=1, space="PSUM") as psattn, \
             tc.tile_pool(name="dram", bufs=1, space="DRAM") as dramp:

            # ---------- weights to SBUF ----------
            kvw_a = wp.tile([CA, 2 * C], dt.bfloat16)
            kvw_b = wp.tile([CB, 2 * C], dt.bfloat16)
            nc.sync.dma_start(kvw_a[:], kvw_t.ap()[0:CA])
            nc.sync.dma_start(kvw_b[:], kvw_t.ap()[CA:C])
            qw_a = wp.tile([CA, C], dt.bfloat16)
            qw_b = wp.tile([CB, C], dt.bfloat16)
            nc.sync.dma_start(qw_a[:], qw_t.ap()[0:CA])
            nc.sync.dma_start(qw_b[:], qw_t.ap()[CA:C])
            qdw_a = wp.tile([CA, 9, C], dt.bfloat16)
            qdw_b = wp.tile([CB, 9, C], dt.bfloat16)
            nc.sync.dma_start(qdw_a[:], qdw_t.ap().rearrange("t k m -> k t m")[0:CA])
            nc.sync.dma_start(qdw_b[:], qdw_t.ap().rearrange("t k m -> k t m")[CA:C])
            dwk_a = wp.tile([CA, 9], dt.float32)
            dwk_u = wp.tile([CA, 9], dt.float32)          # rows 64:128 hold k ch 128:192
            nc.sync.dma_start(dwk_a[:], dwk_t.ap()[0:CA])
            nc.sync.dma_start(dwk_u[CB:CA, :], dwk_t.ap()[CA:C])
            dwv_a = wp.tile([CA, 9], dt.float32)
            dwv_b = wp.tile([CB, 9], dt.float32)
            nc.sync.dma_start(dwv_a[:], dwv_t.ap()[0:CA])
            nc.sync.dma_start(dwv_b[:], dwv_t.ap()[CA:C])
            projw_a = wp.tile([CA, C], dt.bfloat16)
            projw_b = wp.tile([CB, C], dt.bfloat16)
            nc.sync.dma_start(projw_a[:], projw_t.ap()[0:CA])
            nc.sync.dma_start(projw_b[:], projw_t.ap()[CA:C])
            temp_s = wp.tile([HC, HEADS], dt.float32)
            nc.sync.dma_start(temp_s[:], temp_t.ap())

            # ---------- persistent intermediates ----------
            q1a = bigp.tile([CA, PH, PW], dt.bfloat16, tag="q1a")
            q1b = bigp.tile([CB, PH, PW], dt.bfloat16, tag="q1b")
            k1a = bigp.tile([CA, PH, PW], dt.bfloat16, tag="k1a")
            # kv1b: rows 0:64 = v ch 128:192 ("v1b"), rows 64:128 = k ch 128:192 ("k1u")
            kv1b = bigp.tile([CA, PH, PW], dt.bfloat16, tag="kv1b")
            v1a = bigp.tile([CA, PH, PW], dt.bfloat16, tag="v1a")
            qa = bigp.tile([CA, S_IN], dt.bfloat16, tag="qa")
            qb = bigp.tile([CB, S_IN], dt.bfloat16, tag="qb")
            ka = bigp.tile([CA, S_IN], dt.bfloat16, tag="ka")
            # kvb_out: rows 0:64 = v ch 128:192 after dw, rows 64:128 = k ch 128:192 after dw
            kvb_out = bigp.tile([CA, S_IN], dt.bfloat16, tag="kvb_out")
            # va reuses the q1a slot: q1 is dead before the v depthwise runs
            va = bigp.tile([CA, S_IN], dt.bfloat16, tag="q1a")

            def flat(t):
                return t[:].rearrange("p h w -> p (h w)")

            # ---------- phase 1: q1 = q_w @ y (padded grid) ----------
            for c0, c1 in PCH:
                n = c1 - c0
                ya = iop.tile([CA, 512], dt.bfloat16, tag="ld_a")
                yb = iop.tile([CB, 512], dt.bfloat16, tag="ld_b")
                ysrc = y_pad_t.ap().rearrange("c h w -> c (h w)")
                nc.sync.dma_start(ya[:, :n], ysrc[0:CA, c0:c1])
                nc.sync.dma_start(yb[:, :n], ysrc[CA:C, c0:c1])
                pa = psp.tile([CA, 512], dt.float32, tag="mm")
                pb = psp.tile([CB, 512], dt.float32, tag="mm")
                nc.tensor.matmul(pa[:, :n], qw_a[:, 0:CA], ya[:, :n], start=True, stop=False)
                nc.tensor.matmul(pa[:, :n], qw_b[:, 0:CA], yb[:, :n], start=False, stop=True)
                nc.tensor.matmul(pb[:, :n], qw_a[:, CA:C], ya[:, :n], start=True, stop=False)
                nc.tensor.matmul(pb[:, :n], qw_b[:, CA:C], yb[:, :n], start=False, stop=True)
                nc.any.tensor_copy(flat(q1a)[:, c0:c1], pa[:, :n])
                nc.any.tensor_copy(flat(q1b)[:, c0:c1], pb[:, :n])

            # ---------- phase 2: kv1 = kv_w @ x (padded grid) ----------
            # host-permuted output channels:
            #   psum0 = k[0:128]; psum1 = [k 128:192 ; v 128:192]; psum2 = v[0:128]
            for c0, c1 in PCH:
                n = c1 - c0
                xa = iop.tile([CA, 512], dt.bfloat16, tag="ld_a")
                xb = iop.tile([CB, 512], dt.bfloat16, tag="ld_b")
                xsrc = x_pad_t.ap().rearrange("c h w -> c (h w)")
                nc.sync.dma_start(xa[:, :n], xsrc[0:CA, c0:c1])
                nc.sync.dma_start(xb[:, :n], xsrc[CA:C, c0:c1])
                p0 = psp.tile([CA, 512], dt.float32, tag="mm")
                p1 = psp.tile([CA, 512], dt.float32, tag="mm")
                p2 = psp.tile([CA, 512], dt.float32, tag="mm")
                for p, m0 in ((p0, 0), (p1, 128), (p2, 256)):
                    nc.tensor.matmul(p[:, :n], kvw_a[:, m0:m0 + 128], xa[:, :n], start=True, stop=False)
                    nc.tensor.matmul(p[:, :n], kvw_b[:, m0:m0 + 128], xb[:, :n], start=False, stop=True)
                nc.any.tensor_copy(flat(k1a)[:, c0:c1], p0[:, :n])
                nc.any.tensor_copy(flat(kv1b)[:, c0:c1], p1[:, :n])
                nc.any.tensor_copy(flat(v1a)[:, c0:c1], p2[:, :n])

            # ---------- depthwise 3x3 taps (FMA on VectorE) ----------
            def dw(dst, src, wsc, plo, phi):
                first = True
                for t, (ky, kx) in enumerate(taps):
                    shifted = src[plo:phi, ky:ky + HP, kx:kx + W]
                    d = dst[plo:phi].rearrange("p (h w) -> p h w", w=W)
                    if first:
                        nc.vector.tensor_scalar(d, shifted, wsc[plo:phi, t:t + 1], None, Alu.mult)
                        first = False
                    else:
                        nc.vector.scalar_tensor_tensor(
                            d, shifted, wsc[plo:phi, t:t + 1], d, Alu.mult, Alu.add)

            # ---------- phase 3: depthwise on k ----------
            dw(ka, k1a, dwk_a, 0, CA)
            dw(kvb_out, kv1b, dwk_u, CB, CA)

            # ---------- phase 4: q = 9-tap full conv over q1 ----------
            for i in range(S_IN // 512):
                r0 = 4 * i
                pqa = psp.tile([CA, 512], dt.float32, tag="mm")
                pqb = psp.tile([CB, 512], dt.float32, tag="mm")
                for p, m0, mw in ((pqa, 0, CA), (pqb, CA, CB)):
                    for t, (ky, kx) in enumerate(taps):
                        rhs_a = q1a[:, r0 + ky:r0 + ky + 4, kx:kx + W]
                        rhs_b = q1b[:, r0 + ky:r0 + ky + 4, kx:kx + W]
                        nc.tensor.matmul(p[:], qdw_a[:, t, m0:m0 + mw], rhs_a,
                                         start=(t == 0), stop=False)
                        nc.tensor.matmul(p[:], qdw_b[:, t, m0:m0 + mw], rhs_b,
                                         start=False, stop=(t == 8))
                nc.any.tensor_copy(qa[:, 512 * i:512 * (i + 1)], pqa[:])
                nc.any.tensor_copy(qb[:, 512 * i:512 * (i + 1)], pqb[:])

            # ---------- phase 5: sum of squares of q and k rows ----------
            ssq_a = smp.tile([CA, 16], dt.float32)
            ssq_b = smp.tile([CB, 16], dt.float32)
            ssk_a = smp.tile([CA, 16], dt.float32)
            ssk_u = smp.tile([CA, 16], dt.float32)   # rows 64:128 active
            for (srct, plo, phi, dst) in ((qa, 0, CA, ssq_a), (qb, 0, CB, ssq_b),
                                          (ka, 0, CA, ssk_a), (kvb_out, CB, CA, ssk_u)):
                for i in range(16):
                    sq = evp.tile([CA, 512], dt.bfloat16, tag="sqscr")
                    nc.scalar.activation(sq[plo:phi], srct[plo:phi, 512 * i:512 * (i + 1)],
                                         Act.Square, accum_out=dst[plo:phi, i:i + 1])
            ssq = smp.tile([CA, 2], dt.float32)
            ssk = smp.tile([CA, 2], dt.float32)
            nc.vector.tensor_reduce(ssq[:, 0:1], ssq_a[:], mybir.AxisListType.X, Alu.add)
            nc.vector.tensor_reduce(ssq[0:CB, 1:2], ssq_b[:], mybir.AxisListType.X, Alu.add)
            nc.vector.tensor_reduce(ssk[:, 0:1], ssk_a[:], mybir.AxisListType.X, Alu.add)
            nc.vector.tensor_reduce(ssk[CB:CA, 1:2], ssk_u[CB:CA], mybir.AxisListType.X, Alu.add)

            # ---------- phase 6: attn_raw = q @ k^T (contract local spatial) ----------
            attn_pa = psattn.tile([CA, C], dt.float32, tag="attnA")
            attn_pb = psattn.tile([CB, C], dt.float32, tag="attnB")
            NT = S_IN // 128
            for i in range(NT):
                n0 = 128 * i
                qt = tpp.tile([128, C], dt.bfloat16, tag="qt")
                kt = tpp.tile([128, C], dt.bfloat16, tag="kt")
                nc.sync.dma_start_transpose(qt[:, 0:CA], qa[:, n0:n0 + 128])
                nc.sync.dma_start_transpose(qt[:, CA:C], qb[:, n0:n0 + 128])
                nc.sync.dma_start_transpose(kt[:, 0:CA], ka[:, n0:n0 + 128])
                nc.sync.dma_start_transpose(kt[:, CA:C], kvb_out[CB:CA, n0:n0 + 128])
                nc.tensor.matmul(attn_pa[:], qt[:, 0:CA], kt[:],
                                 start=(i == 0), stop=(i == NT - 1),
                                 skip_group_check=True)
                nc.tensor.matmul(attn_pb[:], qt[:, CA:C], kt[:],
                                 start=(i == 0), stop=(i == NT - 1),
                                 skip_group_check=True)

            # ---------- phase 7: pack + pairwise all-reduce ----------
            attn_sa = smp.tile([CA, C], dt.float32)
            attn_sb = smp.tile([CB, C], dt.float32)
            nc.any.tensor_copy(attn_sa[:], attn_pa[:])
            nc.any.tensor_copy(attn_sb[:], attn_pb[:])
            cin = dramp.tile([34, C], dt.float32)
            cout = dramp.tile([34, C], dt.float32)
            for h in range(HEADS):
                src = attn_sa if h < 4 else attn_sb
                r = HC * (h % 4)
                nc.sync.dma_start(cin[0:HC, HC * h:HC * (h + 1)],
                                  src[r:r + HC, HC * h:HC * (h + 1)])
            nc.sync.dma_start(cin[32:33, 0:CA].rearrange("o c -> c o"), ssq[:, 0:1])
            nc.sync.dma_start(cin[32:33, CA:C].rearrange("o c -> c o"), ssq[0:CB, 1:2])
            nc.sync.dma_start(cin[33:34, 0:CA].rearrange("o c -> c o"), ssk[:, 0:1])
            nc.sync.dma_start(cin[33:34, CA:C].rearrange("o c -> c o"), ssk[CB:CA, 1:2])
            nc.gpsimd.collective_compute(
                "AllReduce", Alu.add,
                replica_groups=[[0, 1], [2, 3], [4, 5], [6, 7]],
                ins=[cin[:].opt()], outs=[cout[:].opt()])

            # ---------- phase 3b: depthwise on v (overlaps collective) ----------
            dw(va, v1a, dwv_a, 0, CA)
            dw(kvb_out, kv1b, dwv_b, 0, CB)

            # ---------- phase 8: softmax ----------
            attn_f = smp.tile([HC, HEADS, HC], dt.float32)
            nc.sync.dma_start(attn_f[:], cout[0:HC, :].rearrange("p (h c) -> p h c", h=HEADS))
            fq = smp.tile([HC, HEADS], dt.float32)
            fk = smp.tile([1, C], dt.float32)
            nc.sync.dma_start(fq[:], cout[32:33, :].rearrange("o (h c) -> c (o h)", h=HEADS))
            nc.sync.dma_start(fk[:], cout[33:34, :])
            for f in (fq, fk):
                nc.scalar.sqrt(f[:], f[:])
                nc.vector.tensor_scalar_max(f[:], f[:], 1e-12)
                nc.vector.reciprocal(f[:], f[:])
            # fq *= temperature (per head)
            nc.vector.tensor_tensor(fq[:], fq[:], temp_s[:], Alu.mult)
            # replicate the fk row to 32 partitions (gpsimd is idle post-collective)
            fk32 = smp.tile([HC, C], dt.float32)
            nc.gpsimd.partition_broadcast(fk32[:], fk[:])
            # attn = attn * fq[c,h] * fk[d]
            nc.vector.tensor_tensor(attn_f[:], attn_f[:],
                                    fq[:, :, None].to_broadcast((HC, HEADS, HC)), Alu.mult)
            nc.vector.tensor_tensor(attn_f[:], attn_f[:],
                                    fk32[:].rearrange("p (h c) -> p h c", h=HEADS), Alu.mult)
            # softmax over last dim (32) per head; |logits| <= max|temp| so no max-sub
            ex = smp.tile([HC, HEADS, HC], dt.float32)
            nc.scalar.activation(ex[:], attn_f[:], Act.Exp)
            sm = smp.tile([HC, HEADS], dt.float32)
            nc.vector.tensor_reduce(sm[:], ex[:], mybir.AxisListType.X, Alu.add)
            nc.vector.reciprocal(sm[:], sm[:])
            nc.vector.tensor_tensor(ex[:], ex[:],
                                    sm[:, :, None].to_broadcast((HC, HEADS, HC)), Alu.mult)
            attn_bf = smp.tile([HC, HEADS, HC], dt.bfloat16)
            nc.vector.tensor_copy(attn_bf[:], ex[:])
            # per-head 32x32 transpose (block transpose)
            attn_T = smp.tile([HC, HEADS, HC], dt.bfloat16)
            nc.vector.transpose(attn_T[:].rearrange("p h c -> p (h c)"),
                                attn_bf[:].rearrange("p h c -> p (h c)"))
            # scatter to block-diagonal lhsT tiles (DMA moves across partitions)
            bd_a = smp.tile([CA, CA], dt.bfloat16)
            bd_b = smp.tile([CB, CB], dt.bfloat16)
            nc.vector.memset(bd_a[:], 0.0)
            nc.vector.memset(bd_b[:], 0.0)
            for h in range(HEADS):
                if h < 4:
                    nc.sync.dma_start(bd_a[HC * h:HC * (h + 1), HC * h:HC * (h + 1)],
                                      attn_T[:, h, :])
                else:
                    j = h - 4
                    nc.sync.dma_start(bd_b[HC * j:HC * (j + 1), HC * j:HC * (j + 1)],
                                      attn_T[:, h, :])

            # ---------- phase 9: out_heads = attn @ v ; proj ; residual (fused) ----------
            for i in range(S_IN // 512):
                s0 = 512 * i
                pva = psp.tile([CA, 512], dt.float32, tag="mm")
                pvb = psp.tile([CB, 512], dt.float32, tag="mm")
                nc.tensor.matmul(pva[:], bd_a[:], va[:, s0:s0 + 512], start=True, stop=True)
                nc.tensor.matmul(pvb[:], bd_b[:], kvb_out[0:CB, s0:s0 + 512],
                                 start=True, stop=True)
                oha = evp.tile([CA, 512], dt.bfloat16, tag="oh_a")
                ohb = evp.tile([CB, 512], dt.bfloat16, tag="oh_b")
                nc.any.tensor_copy(oha[:], pva[:])
                nc.any.tensor_copy(ohb[:], pvb[:])
                ppa = psp.tile([CA, 512], dt.float32, tag="mm")
                ppb = psp.tile([CB, 512], dt.float32, tag="mm")
                nc.tensor.matmul(ppa[:], projw_a[:, 0:CA], oha[:], start=True, stop=False)
                nc.tensor.matmul(ppa[:], projw_b[:, 0:CA], ohb[:], start=False, stop=True)
                nc.tensor.matmul(ppb[:], projw_a[:, CA:C], oha[:], start=True, stop=False)
                nc.tensor.matmul(ppb[:], projw_b[:, CA:C], ohb[:], start=False, stop=True)
                xca = iop2.tile([CA, 512], dt.float32, tag="xc_a")
                xcb = iop2.tile([CB, 512], dt.float32, tag="xc_b")
                nc.sync.dma_start(xca[:], x_ctr_t.ap()[0:CA, s0:s0 + 512])
                nc.sync.dma_start(xcb[:], x_ctr_t.ap()[CA:C, s0:s0 + 512])
                ra = evp2.tile([CA, 512], dt.float32, tag="res_a")
                rb = evp2.tile([CB, 512], dt.float32, tag="res_b")
                nc.vector.scalar_tensor_tensor(ra[:], ppa[:], -1.0, xca[:], Alu.mult, Alu.add)
                nc.vector.scalar_tensor_tensor(rb[:], ppb[:], -1.0, xcb[:], Alu.mult, Alu.add)
                nc.sync.dma_start(out_t.ap()[0:CA, s0:s0 + 512], ra[:])
                nc.sync.dma_start(out_t.ap()[CA:C, s0:s0 + 512], rb[:])

    nc.compile()
    return nc


def _host_prep(inputs):
    x = np.asarray(inputs["x"], dtype=np.float32)
    y = np.asarray(inputs["y"], dtype=np.float32)
    kv_w = np.asarray(inputs["kv_w"], dtype=np.float32)[:, :, 0, 0]
    kv_dw = np.asarray(inputs["kv_dw_w"], dtype=np.float32)[:, 0]
    q_w = np.asarray(inputs["q_w"], dtype=np.float32)[:, :, 0, 0]
    q_dw = np.asarray(inputs["q_dw_w"], dtype=np.float32)
    proj_w = np.asarray(inputs["proj_w"], dtype=np.float32)[:, :, 0, 0]
    temp = np.asarray(inputs["temperature"], dtype=np.float32)[:, 0, 0]

    # kv output-channel permutation: [k 0:128 | v 128:192 ; k 128:192 | v 0:128]
    perm = np.concatenate([np.arange(0, 128), np.arange(320, 384),
                           np.arange(128, 192), np.arange(192, 320)])
    kv_wT = np.ascontiguousarray(kv_w[perm].T).astype(bf16)
    q_wT = np.ascontiguousarray(q_w.T).astype(bf16)
    qdw_T = np.ascontiguousarray(
        np.stack([q_dw[:, :, ky, kx].T for ky in range(3) for kx in range(3)])
    ).astype(bf16)
    dw_k = np.ascontiguousarray(kv_dw[:C].reshape(C, 9))
    dw_v = np.ascontiguousarray(kv_dw[C:].reshape(C, 9))
    proj_wT = np.ascontiguousarray(proj_w.T).astype(bf16)
    temp2 = np.ascontiguousarray(np.broadcast_to(temp.reshape(1, HEADS), (HC, HEADS)))

    def shard(arr, b, s):
        r0 = HP * s
        p = np.zeros((C, PH, PW), np.float32)
        lo, hi = max(r0 - 1, 0), min(r0 + HP + 1, H)
        p[:, lo - r0 + 1:hi - r0 + 1, 1:W + 1] = arr[b, :, lo:hi, :]
        return np.ascontiguousarray(p.astype(bf16))

    in_maps = []
    for core in range(NCORES):
        b, s = core // 2, core % 2
        r0 = HP * s
        in_maps.append({
            "x_pad": shard(x, b, s),
            "y_pad": shard(y, b, s),
            "x_ctr": np.ascontiguousarray(
                x[b, :, r0:r0 + HP, :].reshape(C, S_IN)),
            "kv_wT": kv_wT, "q_wT": q_wT, "qdw_T": qdw_T,
            "dw_k": dw_k, "dw_v": dw_v, "proj_wT": proj_wT,
            "temp": temp2,
        })
    return in_maps


LAST_RESULT = None


def kernel(**inputs):
    global LAST_RESULT
    from concourse.bass_utils import run_bass_kernel_spmd

    if "nc" not in _cache:
        _cache["nc"] = _build()
    nc = _cache["nc"]
    in_maps = _host_prep(inputs)
    res = run_bass_kernel_spmd(nc, in_maps, core_ids=list(range(NCORES)))
    LAST_RESULT = res
    out = np.empty((B, C, H, W), np.float32)
    for core in range(NCORES):
        b, s = core // 2, core % 2
        out[b, :, HP * s:HP * (s + 1), :] = \
            res.results[core]["out"].reshape(C, HP, W)
    return out


# revision 11
# speedup vs baseline: 2.1447x; 2.1447x over previous
"""Trainium2 Bass kernel for nn_Attention_57672820850902.

Channel-attention block (XCA-style):
  kv = dwconv3x3(conv1x1(x)); k, v = split(kv)
  q  = conv3x3_full(conv1x1(y))
  q, k l2-normalized per channel row; attn = softmax(q @ k^T * temp) per head
  out = x - conv1x1(attn @ v)

Sharding: 8 cores = 4 batches x 2 spatial halves (64 rows + 1-row halo).
All convs are local to a core; the only cross-core data is a 26KB
pairwise AllReduce carrying per-channel sum-of-squares (for the l2 norm)
and the per-head 32x32 attention logits (contracted over local spatial).

Perf notes:
- Channels (192) are tiled 128+64; every contraction (K) dim is padded to
  128 with zero weight rows -- K=64 matmuls break the PE's LDWEIGHTS
  pipelining (row-group conflicts) and halve throughput.
- The depthwise conv runs as 9 scalar_tensor_tensor FMA taps on VectorE
  over a zero-padded [C, 66, 130] layout; the k-upper/v-upper 64-channel
  halves share one 128-partition tile (kv1b/kvb_out) so no DVE lanes idle.
- q/k are transposed for the QK^T contraction with 512-wide xbar DMA
  transposes (3D-output form, one instr per 512 cols) split across the
  two HWDGE queues (sync + scalar).
"""

import os
import numpy as np
import ml_dtypes

B, C, H, W, HEADS = 4, 192, 128, 128, 6
HC = C // HEADS                      # 32 channels per head
HP = H // 2                          # 64 rows per core
PH, PW = HP + 2, W + 2               # 66 x 130 padded shard
S_PAD = PH * PW                      # 8580
S_IN = HP * W                        # 8192
NCORES = 8
CA, CB = 128, 64                     # channel tile split of 192
CP = 256                             # K-padded channel count

bf16 = ml_dtypes.bfloat16

_cache = {}


def _pad_chunks():
    bounds = list(range(0, S_PAD, 512)) + [S_PAD]
    return list(zip(bounds[:-1], bounds[1:]))


def _build():
    import concourse.bass as bass
    import concourse.mybir as mybir
    import concourse.tile as tile
    from concourse import bacc

    dt = mybir.dt
    Alu = mybir.AluOpType
    Act = mybir.ActivationFunctionType

    nc = bacc.Bacc("TRN2", target_bir_lowering=False, debug=False,
                   num_devices=NCORES)

    # ---- per-core inputs (channel dim host-padded to 256 with zeros) ----
    x_pad_t = nc.dram_tensor("x_pad", [CP, PH, PW], dt.bfloat16, kind="ExternalInput")
    y_pad_t = nc.dram_tensor("y_pad", [CP, PH, PW], dt.bfloat16, kind="ExternalInput")
    x_ctr_t = nc.dram_tensor("x_ctr", [C, S_IN], dt.float32, kind="ExternalInput")
    # ---- weights (same on all cores; K rows host-padded to 256) ----
    # kv_wT columns host-permuted to [k 0:128 | v 128:192 ; k 128:192 | v 0:128]
    kvw_t = nc.dram_tensor("kv_wT", [CP, 2 * C], dt.bfloat16, kind="ExternalInput")
    qw_t = nc.dram_tensor("q_wT", [CP, C], dt.bfloat16, kind="ExternalInput")
    qdw_t = nc.dram_tensor("qdw_T", [9, CP, C], dt.bfloat16, kind="ExternalInput")
    # dw_all rows: [0:128]=k 0:128 | [128:192]=v 128:192 ; [192:256]=k 128:192 | [256:384]=v 0:128
    dw_t = nc.dram_tensor("dw_all", [384, 9], dt.float32, kind="ExternalInput")
    projw_t = nc.dram_tensor("proj_wT", [CP, C], dt.bfloat16, kind="ExternalInput")
    temp_t = nc.dram_tensor("temp", [HC, HEADS], dt.float32, kind="ExternalInput")
    out_t = nc.dram_tensor("out", [C, S_IN], dt.float32, kind="ExternalOutput")

    PCH = _pad_chunks()
    taps = [(ky, kx) for ky in range(3) for kx in range(3)]
    NCH = S_IN // 512                # 16 inner chunks

    with tile.TileContext(nc) as tc:
        with tc.tile_pool(name="w", bufs=1) as wp, \
             tc.tile_pool(name="big", bufs=1) as bigp, \
             tc.tile_pool(name="io", bufs=3) as iop, \
             tc.tile_pool(name="io2", bufs=2) as iop2, \
             tc.tile_pool(name="tp", bufs=2) as tpp, \
             tc.tile_pool(name="ev", bufs=2) as evp, \
             tc.tile_pool(name="oh", bufs=3) as ohp, \
             tc.tile_pool(name="small", bufs=1) as smp, \
             tc.tile_pool(name="ps", bufs=6, space="PSUM") as psp, \
             tc.tile_pool(name="psattn", bufs=1, space="PSUM") as psattn, \
             tc.tile_pool(name="dram", bufs=1, space="DRAM") as dramp:

            # ---------- weights to SBUF ----------
            kvw_a = wp.tile([CA, 2 * C], dt.bfloat16)
            kvw_b = wp.tile([CA, 2 * C], dt.bfloat16)
            nc.sync.dma_start(kvw_a[:], kvw_t.ap()[0:CA])
            nc.sync.dma_start(kvw_b[:], kvw_t.ap()[CA:CP])
            qw_a = wp.tile([CA, C], dt.bfloat16)
            qw_b = wp.tile([CA, C], dt.bfloat16)
            nc.sync.dma_start(qw_a[:], qw_t.ap()[0:CA])
            nc.sync.dma_start(qw_b[:], qw_t.ap()[CA:CP])
            qdw_a = wp.tile([CA, 9, C], dt.bfloat16)
            qdw_b = wp.tile([CA, 9, C], dt.bfloat16)
            nc.sync.dma_start(qdw_a[:], qdw_t.ap().rearrange("t k m -> k t m")[0:CA])
            nc.sync.dma_start(qdw_b[:], qdw_t.ap().rearrange("t k m -> k t m")[CA:CP])
            dwk_a = wp.tile([CA, 9], dt.float32)
            dw_mix = wp.tile([CA, 9], dt.float32)
            dwv_a = wp.tile([CA, 9], dt.float32)
            nc.sync.dma_start(dwk_a[:], dw_t.ap()[0:128])
            nc.sync.dma_start(dw_mix[:], dw_t.ap()[128:256])
            nc.sync.dma_start(dwv_a[:], dw_t.ap()[256:384])
            projw_a = wp.tile([CA, C], dt.bfloat16)
            projw_b = wp.tile([CA, C], dt.bfloat16)
            nc.sync.dma_start(projw_a[:], projw_t.ap()[0:CA])
            nc.sync.dma_start(projw_b[:], projw_t.ap()[CA:CP])
            temp_s = wp.tile([HC, HEADS], dt.float32)
            nc.sync.dma_start(temp_s[:], temp_t.ap())

            # ---------- persistent intermediates ----------
            k1a = bigp.tile([CA, PH, PW], dt.bfloat16, tag="k1a")
            # kv1b: rows 0:64 = v ch 128:192 ("v1b"), rows 64:128 = k ch 128:192 ("k1u")
            kv1b = bigp.tile([CA, PH, PW], dt.bfloat16, tag="kv1b")
            v1a = bigp.tile([CA, PH, PW], dt.bfloat16, tag="v1a")
            q1a = bigp.tile([CA, PH, PW], dt.bfloat16, tag="q1a")
            q1b = bigp.tile([CA, PH, PW], dt.bfloat16, tag="q1b")  # rows 64:128 zeroed
            qa = bigp.tile([CA, S_IN], dt.bfloat16, tag="qa")
            qb = bigp.tile([CB, S_IN], dt.bfloat16, tag="qb")
            ka = bigp.tile([CA, S_IN], dt.bfloat16, tag="ka")
            # kvb_out: rows 0:64 = v ch 128:192 dw'd, rows 64:128 = k ch 128:192 dw'd
            kvb_out = bigp.tile([CA, S_IN], dt.bfloat16, tag="kvb_out")
            # va reuses the k1a slot (k1a dead after the k depthwise)
            va = bigp.tile([CA, S_IN], dt.bfloat16, tag="k1a")

            nc.vector.memset(q1b[CB:CA, :, :], 0.0)

            attn_pa = psattn.tile([CA, C], dt.float32, tag="attnA")
            attn_pb = psattn.tile([CB, C], dt.float32, tag="attnB")

            def flat(t):
                return t[:].rearrange("p h w -> p (h w)")

            # ---------- phase 1: kv1 = kv_w @ x (padded grid) ----------
            #   psum0 = k[0:128]; psum1 = [v 128:192 ; k 128:192]; psum2 = v[0:128]
            for c0, c1 in PCH:
                n = c1 - c0
                xa = iop.tile([CA, 512], dt.bfloat16, tag="ld_a")
                xb = iop.tile([CA, 512], dt.bfloat16, tag="ld_b")
                xsrc = x_pad_t.ap().rearrange("c h w -> c (h w)")
                nc.sync.dma_start(xa[:, :n], xsrc[0:CA, c0:c1])
                nc.sync.dma_start(xb[:, :n], xsrc[CA:CP, c0:c1])
                p0 = psp.tile([CA, 512], dt.float32, tag="mm")
                p1 = psp.tile([CA, 512], dt.float32, tag="mm")
                p2 = psp.tile([CA, 512], dt.float32, tag="mm")
                for p, m0 in ((p0, 0), (p1, 128), (p2, 256)):
                    nc.tensor.matmul(p[:, :n], kvw_a[:, m0:m0 + 128], xa[:, :n], start=True, stop=False)
                    nc.tensor.matmul(p[:, :n], kvw_b[:, m0:m0 + 128], xb[:, :n], start=False, stop=True)
                nc.any.tensor_copy(flat(k1a)[:, c0:c1], p0[:, :n])
                nc.any.tensor_copy(flat(kv1b)[:, c0:c1], p1[:, :n])
                nc.any.tensor_copy(flat(v1a)[:, c0:c1], p2[:, :n])

            # ---------- depthwise 3x3 taps (FMA on VectorE) ----------
            def dw(dst, src, wsc):
                first = True
                for t, (ky, kx) in enumerate(taps):
                    shifted = src[:, ky:ky + HP, kx:kx + W]
                    d = dst[:].rearrange("p (h w) -> p h w", w=W)
                    if first:
                        nc.vector.tensor_scalar(d, shifted, wsc[:, t:t + 1], None, Alu.mult)
                        first = False
                    else:
                        nc.vector.scalar_tensor_tensor(
                            d, shifted, wsc[:, t:t + 1], d, Alu.mult, Alu.add)

            # ---------- phase 2: depthwise on k (and v upper half) ----------
            dw(ka, k1a, dwk_a)
            dw(kvb_out, kv1b, dw_mix)

            # ---------- phase 3: q1 = q_w @ y ----------
            for c0, c1 in PCH:
                n = c1 - c0
                ya = iop.tile([CA, 512], dt.bfloat16, tag="ld_a")
                yb = iop.tile([CA, 512], dt.bfloat16, tag="ld_b")
                ysrc = y_pad_t.ap().rearrange("c h w -> c (h w)")
                nc.sync.dma_start(ya[:, :n], ysrc[0:CA, c0:c1])
                nc.sync.dma_start(yb[:, :n], ysrc[CA:CP, c0:c1])
                pa = psp.tile([CA, 512], dt.float32, tag="mm")
                pb = psp.tile([CB, 512], dt.float32, tag="mm")
                nc.tensor.matmul(pa[:, :n], qw_a[:, 0:CA], ya[:, :n], start=True, stop=False)
                nc.tensor.matmul(pa[:, :n], qw_b[:, 0:CA], yb[:, :n], start=False, stop=True)
                nc.tensor.matmul(pb[:, :n], qw_a[:, CA:C], ya[:, :n], start=True, stop=False)
                nc.tensor.matmul(pb[:, :n], qw_b[:, CA:C], yb[:, :n], start=False, stop=True)
                nc.any.tensor_copy(flat(q1a)[:, c0:c1], pa[:, :n])
                nc.any.tensor_copy(flat(q1b)[0:CB, c0:c1], pb[:, :n])

            # ---------- phase 4: q = 9-tap full conv; transposes + QK^T interleaved ----------
            for i in range(NCH):
                r0 = 4 * i
                pqa = psp.tile([CA, 512], dt.float32, tag="mm")
                pqb = psp.tile([CB, 512], dt.float32, tag="mm")
                for p, m0, mw in ((pqa, 0, CA), (pqb, CA, CB)):
                    for t, (ky, kx) in enumerate(taps):
                        rhs_a = q1a[:, r0 + ky:r0 + ky + 4, kx:kx + W]
                        rhs_b = q1b[:, r0 + ky:r0 + ky + 4, kx:kx + W]
                        nc.tensor.matmul(p[:], qdw_a[:, t, m0:m0 + mw], rhs_a,
                                         start=(t == 0), stop=False)
                        nc.tensor.matmul(p[:], qdw_b[:, t, m0:m0 + mw], rhs_b,
                                         start=False, stop=(t == 8))
                s0 = 512 * i
                nc.any.tensor_copy(qa[:, s0:s0 + 512], pqa[:])
                nc.any.tensor_copy(qb[:, s0:s0 + 512], pqb[:])
                qt4 = tpp.tile([128, 4, C], dt.bfloat16, tag="qt")
                nc.sync.dma_start_transpose(qt4[:, :, 0:CA], qa[:, s0:s0 + 512])
                nc.scalar.dma_start_transpose(qt4[:, :, CA:C], qb[:, s0:s0 + 512])
                kt4 = tpp.tile([128, 4, C], dt.bfloat16, tag="kt")
                nc.scalar.dma_start_transpose(kt4[:, :, 0:CA], ka[:, s0:s0 + 512])
                nc.sync.dma_start_transpose(kt4[:, :, CA:C], kvb_out[CB:CA, s0:s0 + 512])
                for j in range(4):
                    nc.tensor.matmul(attn_pa[:], qt4[:, j, 0:CA], kt4[:, j, :],
                                     start=(i == 0 and j == 0),
                                     stop=(i == NCH - 1 and j == 3),
                                     skip_group_check=True)
                    nc.tensor.matmul(attn_pb[:], qt4[:, j, CA:C], kt4[:, j, :],
                                     start=(i == 0 and j == 0),
                                     stop=(i == NCH - 1 and j == 3),
                                     skip_group_check=True)

            # ---------- phase 5: sum of squares of q and k rows ----------
            ssq_a = smp.tile([CA, 8], dt.float32)
            ssq_b = smp.tile([CB, 8], dt.float32)
            ssk_a = smp.tile([CA, 8], dt.float32)
            ssk_u = smp.tile([CA, 8], dt.float32)   # rows 64:128 active
            for (srct, plo, phi, dst) in ((qa, 0, CA, ssq_a), (qb, 0, CB, ssq_b),
                                          (ka, 0, CA, ssk_a), (kvb_out, CB, CA, ssk_u)):
                for i in range(8):
                    sq = evp.tile([CA, 1024], dt.bfloat16, tag="sqscr")
                    nc.scalar.activation(sq[plo:phi], srct[plo:phi, 1024 * i:1024 * (i + 1)],
                                         Act.Square, accum_out=dst[plo:phi, i:i + 1])
            ssq = smp.tile([CA, 2], dt.float32)
            ssk = smp.tile([CA, 2], dt.float32)
            nc.vector.tensor_reduce(ssq[:, 0:1], ssq_a[:], mybir.AxisListType.X, Alu.add)
            nc.vector.tensor_reduce(ssq[0:CB, 1:2], ssq_b[:], mybir.AxisListType.X, Alu.add)
            nc.vector.tensor_reduce(ssk[:, 0:1], ssk_a[:], mybir.AxisListType.X, Alu.add)
            nc.vector.tensor_reduce(ssk[CB:CA, 1:2], ssk_u[CB:CA], mybir.AxisListType.X, Alu.add)

            # ---------- phase 6: pack + pairwise all-reduce ----------
            attn_sa = smp.tile([CA, C], dt.float32)
            attn_sb = smp.tile([CB, C], dt.float32)
            nc.any.tensor_copy(attn_sa[:], attn_pa[:])
            nc.any.tensor_copy(attn_sb[:], attn_pb[:])
            cin = dramp.tile([34, C], dt.float32)
            cout = dramp.tile([34, C], dt.float32)
            for h in range(HEADS):
                src = attn_sa if h < 4 else attn_sb
                r = HC * (h % 4)
                nc.sync.dma_start(cin[0:HC, HC * h:HC * (h + 1)],
                                  src[r:r + HC, HC * h:HC * (h + 1)])
            nc.sync.dma_start(cin[32:33, 0:CA].rearrange("o c -> c o"), ssq[:, 0:1])
            nc.sync.dma_start(cin[32:33, CA:C].rearrange("o c -> c o"), ssq[0:CB, 1:2])
            nc.sync.dma_start(cin[33:34, 0:CA].rearrange("o c -> c o"), ssk[:, 0:1])
            nc.sync.dma_start(cin[33:34, CA:C].rearrange("o c -> c o"), ssk[CB:CA, 1:2])
            nc.gpsimd.collective_compute(
                "AllReduce", Alu.add,
                replica_groups=[[0, 1], [2, 3], [4, 5], [6, 7]],
                ins=[cin[:].opt()], outs=[cout[:].opt()])

            # ---------- phase 2b: depthwise on v lower half (overlaps collective) ----------
            dw(va, v1a, dwv_a)

            # ---------- phase 7: softmax ----------
            attn_f = smp.tile([HC, HEADS, HC], dt.float32)
            nc.sync.dma_start(attn_f[:], cout[0:HC, :].rearrange("p (h c) -> p h c", h=HEADS))
            fq = smp.tile([HC, HEADS], dt.float32)
            fk = smp.tile([1, C], dt.float32)
            nc.sync.dma_start(fq[:], cout[32:33, :].rearrange("o (h c) -> c (o h)", h=HEADS))
            nc.sync.dma_start(fk[:], cout[33:34, :])
            for f in (fq, fk):
                nc.scalar.sqrt(f[:], f[:])
                nc.vector.tensor_scalar_max(f[:], f[:], 1e-12)
                nc.vector.reciprocal(f[:], f[:])
            nc.vector.tensor_tensor(fq[:], fq[:], temp_s[:], Alu.mult)
            fk32 = smp.tile([HC, C], dt.float32)
            nc.gpsimd.partition_broadcast(fk32[:], fk[:])
            nc.vector.tensor_tensor(attn_f[:], attn_f[:],
                                    fq[:, :, None].to_broadcast((HC, HEADS, HC)), Alu.mult)
            nc.vector.tensor_tensor(attn_f[:], attn_f[:],
                                    fk32[:].rearrange("p (h c) -> p h c", h=HEADS), Alu.mult)
            ex = smp.tile([HC, HEADS, HC], dt.float32)
            nc.scalar.activation(ex[:], attn_f[:], Act.Exp)
            sm = smp.tile([HC, HEADS], dt.float32)
            nc.vector.tensor_reduce(sm[:], ex[:], mybir.AxisListType.X, Alu.add)
            nc.vector.reciprocal(sm[:], sm[:])
            nc.vector.tensor_tensor(ex[:], ex[:],
                                    sm[:, :, None].to_broadcast((HC, HEADS, HC)), Alu.mult)
            attn_bf = smp.tile([HC, HEADS, HC], dt.bfloat16)
            nc.vector.tensor_copy(attn_bf[:], ex[:])
            attn_T = smp.tile([HC, HEADS, HC], dt.bfloat16)
            nc.vector.transpose(attn_T[:].rearrange("p h c -> p (h c)"),
                                attn_bf[:].rearrange("p h c -> p (h c)"))
            bd_a = smp.tile([CA, CA], dt.bfloat16)
            bd_b = smp.tile([CA, CB], dt.bfloat16)   # K-padded: rows 64:128 zero
            nc.vector.memset(bd_a[:], 0.0)
            nc.vector.memset(bd_b[:], 0.0)
            for h in range(HEADS):
                if h < 4:
                    nc.sync.dma_start(bd_a[HC * h:HC * (h + 1), HC * h:HC * (h + 1)],
                                      attn_T[:, h, :])
                else:
                    j = h - 4
                    nc.sync.dma_start(bd_b[HC * j:HC * (j + 1), HC * j:HC * (j + 1)],
                                      attn_T[:, h, :])

            # ---------- phase 8: out_heads = attn @ v ; proj ; residual (fused) ----------
            for i in range(NCH):
                s0 = 512 * i
                pva = psp.tile([CA, 512], dt.float32, tag="mm")
                pvb = psp.tile([CB, 512], dt.float32, tag="mm")
                nc.tensor.matmul(pva[:], bd_a[:], va[:, s0:s0 + 512], start=True, stop=True)
                # rhs rows 64:128 hold dw'd k-upper; bd_b zero rows cancel them
                nc.tensor.matmul(pvb[:], bd_b[:], kvb_out[:, s0:s0 + 512],
                                 start=True, stop=True)
                oha = ohp.tile([CA, 512], dt.bfloat16, tag="oh_a")
                ohb = ohp.tile([CA, 512], dt.bfloat16, tag="oh_b")
                nc.gpsimd.memset(ohb[CB:CA, :], 0.0)
                nc.any.tensor_copy(oha[:], pva[:])
                nc.any.tensor_copy(ohb[0:CB, :], pvb[:])
                ppa = psp.tile([CA, 512], dt.float32, tag="mm")
                ppb = psp.tile([CB, 512], dt.float32, tag="mm")
                nc.tensor.matmul(ppa[:], projw_a[:, 0:CA], oha[:], start=True, stop=False)
                nc.tensor.matmul(ppa[:], projw_b[:, 0:CA], ohb[:], start=False, stop=True)
                nc.tensor.matmul(ppb[:], projw_a[:, CA:C], oha[:], start=True, stop=False)
                nc.tensor.matmul(ppb[:], projw_b[:, CA:C], ohb[:], start=False, stop=True)
                xca = iop2.tile([CA, 512], dt.float32, tag="xc_a")
                xcb = iop2.tile([CB, 512], dt.float32, tag="xc_b")
                nc.sync.dma_start(xca[:], x_ctr_t.ap()[0:CA, s0:s0 + 512])
                nc.sync.dma_start(xcb[:], x_ctr_t.ap()[CA:C, s0:s0 + 512])
                nc.vector.scalar_tensor_tensor(xca[:], ppa[:], -1.0, xca[:], Alu.mult, Alu.add)
                nc.vector.scalar_tensor_tensor(xcb[:], ppb[:], -1.0, xcb[:], Alu.mult, Alu.add)
                nc.sync.dma_start(out_t.ap()[0:CA, s0:s0 + 512], xca[:])
                nc.sync.dma_start(out_t.ap()[CA:C, s0:s0 + 512], xcb[:])

    nc.compile()
    return nc


def _host_prep(inputs):
    x = np.asarray(inputs["x"], dtype=np.float32)
    y = np.asarray(inputs["y"], dtype=np.float32)
    kv_w = np.asarray(inputs["kv_w"], dtype=np.float32)[:, :, 0, 0]
    kv_dw = np.asarray(inputs["kv_dw_w"], dtype=np.float32)[:, 0]
    q_w = np.asarray(inputs["q_w"], dtype=np.float32)[:, :, 0, 0]
    q_dw = np.asarray(inputs["q_dw_w"], dtype=np.float32)
    proj_w = np.asarray(inputs["proj_w"], dtype=np.float32)[:, :, 0, 0]
    temp = np.asarray(inputs["temperature"], dtype=np.float32)[:, 0, 0]

    def kpad(a):  # [192, M] -> [256, M] with zero rows
        return np.concatenate([a, np.zeros((CP - C, a.shape[1]), a.dtype)], 0)

    # kv output-channel permutation: [k 0:128 | v 128:192 ; k 128:192 | v 0:128]
    perm = np.concatenate([np.arange(0, 128), np.arange(320, 384),
                           np.arange(128, 192), np.arange(192, 320)])
    kv_wT = np.ascontiguousarray(kpad(kv_w[perm].T)).astype(bf16)
    q_wT = np.ascontiguousarray(kpad(q_w.T)).astype(bf16)
    qdw_T = np.ascontiguousarray(
        np.stack([kpad(q_dw[:, :, ky, kx].T) for ky in range(3) for kx in range(3)])
    ).astype(bf16)
    kdw = kv_dw[:C].reshape(C, 9)
    vdw = kv_dw[C:].reshape(C, 9)
    dw_all = np.ascontiguousarray(np.concatenate(
        [kdw[0:128], vdw[128:192], kdw[128:192], vdw[0:128]], 0))
    proj_wT = np.ascontiguousarray(kpad(proj_w.T)).astype(bf16)
    temp2 = np.ascontiguousarray(np.broadcast_to(temp.reshape(1, HEADS), (HC, HEADS)))

    def shard(arr, b, s):
        r0 = HP * s
        p = np.zeros((CP, PH, PW), np.float32)
        lo, hi = max(r0 - 1, 0), min(r0 + HP + 1, H)
        p[:C, lo - r0 + 1:hi - r0 + 1, 1:W + 1] = arr[b, :, lo:hi, :]
        return np.ascontiguousarray(p.astype(bf16))

    in_maps = []
    for core in range(NCORES):
        b, s = core // 2, core % 2
        r0 = HP * s
        in_maps.append({
            "x_pad": shard(x, b, s),
            "y_pad": shard(y, b, s),
            "x_ctr": np.ascontiguousarray(
                x[b, :, r0:r0 + HP, :].reshape(C, S_IN)),
            "kv_wT": kv_wT, "q_wT": q_wT, "qdw_T": qdw_T,
            "dw_all": dw_all, "proj_wT": proj_wT,
            "temp": temp2,
        })
    return in_maps


LAST_RESULT = None


def kernel(**inputs):
    global LAST_RESULT
    from concourse.bass_utils import run_bass_kernel_spmd

    if "nc" not in _cache:
        _cache["nc"] = _build()
    nc = _cache["nc"]
    in_maps = _host_prep(inputs)
    res = run_bass_kernel_spmd(nc, in_maps, core_ids=list(range(NCORES)))
    LAST_RESULT = res
    out = np.empty((B, C, H, W), np.float32)
    for core in range(NCORES):
        b, s = core // 2, core % 2
        out[b, :, HP * s:HP * (s + 1), :] = \
            res.results[core]["out"].reshape(C, HP, W)
    return out


# revision 12
# speedup vs baseline: 2.1735x; 1.0134x over previous
"""Trainium2 Bass kernel for nn_Attention_57672820850902.

Channel-attention block (XCA-style):
  kv = dwconv3x3(conv1x1(x)); k, v = split(kv)
  q  = conv3x3_full(conv1x1(y))
  q, k l2-normalized per channel row; attn = softmax(q @ k^T * temp) per head
  out = x - conv1x1(attn @ v)

Sharding: 8 cores = 4 batches x 2 spatial halves (64 rows + 1-row halo).
All convs are local to a core; the only cross-core data is a 26KB
pairwise AllReduce carrying per-channel sum-of-squares (for the l2 norm)
and the per-head 32x32 attention logits (contracted over local spatial).

Perf notes:
- Channels (192) are tiled 128+64; every contraction (K) dim is padded to
  128 with zero weight rows -- K=64 matmuls break the PE's LDWEIGHTS
  pipelining (row-group conflicts) and halve throughput.
- The depthwise conv runs as 9 scalar_tensor_tensor FMA taps on VectorE
  over a zero-padded [C, 66, 130] layout; the k-upper/v-upper 64-channel
  halves share one 128-partition tile (kv1b/kvb_out) so no DVE lanes idle.
- q/k are transposed for the QK^T contraction with 512-wide xbar DMA
  transposes (3D-output form, one instr per 512 cols) split across the
  two HWDGE queues (sync + scalar).
"""

import os
import numpy as np
import ml_dtypes

B, C, H, W, HEADS = 4, 192, 128, 128, 6
HC = C // HEADS                      # 32 channels per head
HP = H // 2                          # 64 rows per core
PH, PW = HP + 2, W + 2               # 66 x 130 padded shard
S_PAD = PH * PW                      # 8580
S_IN = HP * W                        # 8192
NCORES = 8
CA, CB = 128, 64                     # channel tile split of 192
CP = 256                             # K-padded channel count

bf16 = ml_dtypes.bfloat16

_cache = {}


def _pad_chunks():
    bounds = list(range(0, S_PAD, 512)) + [S_PAD]
    return list(zip(bounds[:-1], bounds[1:]))


def _build():
    import concourse.bass as bass
    import concourse.mybir as mybir
    import concourse.tile as tile
    from concourse import bacc

    dt = mybir.dt
    Alu = mybir.AluOpType
    Act = mybir.ActivationFunctionType

    nc = bacc.Bacc("TRN2", target_bir_lowering=False, debug=False,
                   num_devices=NCORES)

    # ---- per-core inputs (channel dim host-padded to 256 with zeros) ----
    x_pad_t = nc.dram_tensor("x_pad", [CP, PH, PW], dt.bfloat16, kind="ExternalInput")
    y_pad_t = nc.dram_tensor("y_pad", [CP, PH, PW], dt.bfloat16, kind="ExternalInput")
    x_ctr_t = nc.dram_tensor("x_ctr", [C, S_IN], dt.float32, kind="ExternalInput")
    # ---- weights (same on all cores; K rows host-padded to 256) ----
    # kv_wT columns host-permuted to [k 0:128 | v 128:192 ; k 128:192 | v 0:128]
    kvw_t = nc.dram_tensor("kv_wT", [CP, 2 * C], dt.bfloat16, kind="ExternalInput")
    qw_t = nc.dram_tensor("q_wT", [CP, C], dt.bfloat16, kind="ExternalInput")
    qdw_t = nc.dram_tensor("qdw_T", [9, CP, C], dt.bfloat16, kind="ExternalInput")
    # dw_all rows: [0:128]=k 0:128 | [128:192]=v 128:192 ; [192:256]=k 128:192 | [256:384]=v 0:128
    dw_t = nc.dram_tensor("dw_all", [384, 9], dt.float32, kind="ExternalInput")
    projw_t = nc.dram_tensor("proj_wT", [CP, C], dt.bfloat16, kind="ExternalInput")
    temp_t = nc.dram_tensor("temp", [HC, HEADS], dt.float32, kind="ExternalInput")
    out_t = nc.dram_tensor("out", [C, S_IN], dt.float32, kind="ExternalOutput")

    PCH = _pad_chunks()
    taps = [(ky, kx) for ky in range(3) for kx in range(3)]
    NCH = S_IN // 512                # 16 inner chunks

    with tile.TileContext(nc) as tc:
        with tc.tile_pool(name="w", bufs=1) as wp, \
             tc.tile_pool(name="big", bufs=1) as bigp, \
             tc.tile_pool(name="io", bufs=2) as iop, \
             tc.tile_pool(name="io2", bufs=2) as iop2, \
             tc.tile_pool(name="tp", bufs=4) as tpp, \
             tc.tile_pool(name="ev", bufs=2) as evp, \
             tc.tile_pool(name="oh", bufs=2) as ohp, \
             tc.tile_pool(name="small", bufs=1) as smp, \
             tc.tile_pool(name="ps", bufs=6, space="PSUM") as psp, \
             tc.tile_pool(name="psattn", bufs=1, space="PSUM") as psattn, \
             tc.tile_pool(name="dram", bufs=1, space="DRAM") as dramp:

            # ---------- weights to SBUF ----------
            kvw_a = wp.tile([CA, 2 * C], dt.bfloat16)
            kvw_b = wp.tile([CA, 2 * C], dt.bfloat16)
            nc.sync.dma_start(kvw_a[:], kvw_t.ap()[0:CA])
            nc.sync.dma_start(kvw_b[:], kvw_t.ap()[CA:CP])
            qw_a = wp.tile([CA, C], dt.bfloat16)
            qw_b = wp.tile([CA, C], dt.bfloat16)
            nc.sync.dma_start(qw_a[:], qw_t.ap()[0:CA])
            nc.sync.dma_start(qw_b[:], qw_t.ap()[CA:CP])
            qdw_a = wp.tile([CA, 9, C], dt.bfloat16)
            qdw_b = wp.tile([CA, 9, C], dt.bfloat16)
            nc.sync.dma_start(qdw_a[:], qdw_t.ap().rearrange("t k m -> k t m")[0:CA])
            nc.sync.dma_start(qdw_b[:], qdw_t.ap().rearrange("t k m -> k t m")[CA:CP])
            dwk_a = wp.tile([CA, 9], dt.float32)
            dw_mix = wp.tile([CA, 9], dt.float32)
            dwv_a = wp.tile([CA, 9], dt.float32)
            nc.sync.dma_start(dwk_a[:], dw_t.ap()[0:128])
            nc.sync.dma_start(dw_mix[:], dw_t.ap()[128:256])
            nc.sync.dma_start(dwv_a[:], dw_t.ap()[256:384])
            projw_a = wp.tile([CA, C], dt.bfloat16)
            projw_b = wp.tile([CA, C], dt.bfloat16)
            nc.sync.dma_start(projw_a[:], projw_t.ap()[0:CA])
            nc.sync.dma_start(projw_b[:], projw_t.ap()[CA:CP])
            temp_s = wp.tile([HC, HEADS], dt.float32)
            nc.sync.dma_start(temp_s[:], temp_t.ap())

            # ---------- persistent intermediates ----------
            k1a = bigp.tile([CA, PH, PW], dt.bfloat16, tag="k1a")
            # kv1b: rows 0:64 = v ch 128:192 ("v1b"), rows 64:128 = k ch 128:192 ("k1u")
            kv1b = bigp.tile([CA, PH, PW], dt.bfloat16, tag="kv1b")
            v1a = bigp.tile([CA, PH, PW], dt.bfloat16, tag="v1a")
            q1a = bigp.tile([CA, PH, PW], dt.bfloat16, tag="q1a")
            q1b = bigp.tile([CA, PH, PW], dt.bfloat16, tag="q1b")  # rows 64:128 zeroed
            qt_full = bigp.tile([128, 64, C], dt.bfloat16, tag="qt_full")
            ka = bigp.tile([CA, S_IN], dt.bfloat16, tag="ka")
            # kvb_out: rows 0:64 = v ch 128:192 dw'd, rows 64:128 = k ch 128:192 dw'd
            kvb_out = bigp.tile([CA, S_IN], dt.bfloat16, tag="kvb_out")
            # va reuses the k1a slot (k1a dead after the k depthwise)
            va = bigp.tile([CA, S_IN], dt.bfloat16, tag="k1a")

            nc.vector.memset(q1b[CB:CA, :, :], 0.0)

            attn_pa = psattn.tile([CA, C], dt.float32, tag="attnA")
            attn_pb = psattn.tile([CB, C], dt.float32, tag="attnB")

            def flat(t):
                return t[:].rearrange("p h w -> p (h w)")

            # ---------- phase 1: kv1 = kv_w @ x (padded grid) ----------
            #   psum0 = k[0:128]; psum1 = [v 128:192 ; k 128:192]; psum2 = v[0:128]
            for c0, c1 in PCH:
                n = c1 - c0
                xa = iop.tile([CA, 512], dt.bfloat16, tag="ld_a")
                xb = iop.tile([CA, 512], dt.bfloat16, tag="ld_b")
                xsrc = x_pad_t.ap().rearrange("c h w -> c (h w)")
                nc.sync.dma_start(xa[:, :n], xsrc[0:CA, c0:c1])
                nc.sync.dma_start(xb[:, :n], xsrc[CA:CP, c0:c1])
                p0 = psp.tile([CA, 512], dt.float32, tag="mm")
                p1 = psp.tile([CA, 512], dt.float32, tag="mm")
                p2 = psp.tile([CA, 512], dt.float32, tag="mm")
                for p, m0 in ((p0, 0), (p1, 128), (p2, 256)):
                    nc.tensor.matmul(p[:, :n], kvw_a[:, m0:m0 + 128], xa[:, :n], start=True, stop=False)
                    nc.tensor.matmul(p[:, :n], kvw_b[:, m0:m0 + 128], xb[:, :n], start=False, stop=True)
                nc.any.tensor_copy(flat(k1a)[:, c0:c1], p0[:, :n])
                nc.any.tensor_copy(flat(kv1b)[:, c0:c1], p1[:, :n])
                nc.any.tensor_copy(flat(v1a)[:, c0:c1], p2[:, :n])

            # ---------- depthwise 3x3 taps (FMA on VectorE) ----------
            def dw(dst, src, wsc):
                first = True
                for t, (ky, kx) in enumerate(taps):
                    shifted = src[:, ky:ky + HP, kx:kx + W]
                    d = dst[:].rearrange("p (h w) -> p h w", w=W)
                    if first:
                        nc.vector.tensor_scalar(d, shifted, wsc[:, t:t + 1], None, Alu.mult)
                        first = False
                    else:
                        nc.vector.scalar_tensor_tensor(
                            d, shifted, wsc[:, t:t + 1], d, Alu.mult, Alu.add)

            # ---------- phase 2: depthwise on k (and v upper half) ----------
            dw(ka, k1a, dwk_a)
            dw(kvb_out, kv1b, dw_mix)
            dw(va, v1a, dwv_a)

            # ---------- phase 3: q1 = q_w @ y ----------
            for c0, c1 in PCH:
                n = c1 - c0
                ya = iop.tile([CA, 512], dt.bfloat16, tag="ld_a")
                yb = iop.tile([CA, 512], dt.bfloat16, tag="ld_b")
                ysrc = y_pad_t.ap().rearrange("c h w -> c (h w)")
                nc.sync.dma_start(ya[:, :n], ysrc[0:CA, c0:c1])
                nc.sync.dma_start(yb[:, :n], ysrc[CA:CP, c0:c1])
                pa = psp.tile([CA, 512], dt.float32, tag="mm")
                pb = psp.tile([CB, 512], dt.float32, tag="mm")
                nc.tensor.matmul(pa[:, :n], qw_a[:, 0:CA], ya[:, :n], start=True, stop=False)
                nc.tensor.matmul(pa[:, :n], qw_b[:, 0:CA], yb[:, :n], start=False, stop=True)
                nc.tensor.matmul(pb[:, :n], qw_a[:, CA:C], ya[:, :n], start=True, stop=False)
                nc.tensor.matmul(pb[:, :n], qw_b[:, CA:C], yb[:, :n], start=False, stop=True)
                nc.any.tensor_copy(flat(q1a)[:, c0:c1], pa[:, :n])
                nc.any.tensor_copy(flat(q1b)[0:CB, c0:c1], pb[:, :n])

            # ---------- phase 4: q = 9-tap full conv; evac -> transpose to qt_full ----------
            ssq_a = smp.tile([CA, 16], dt.float32)
            ssq_b = smp.tile([CB, 16], dt.float32)
            for i in range(NCH):
                r0 = 4 * i
                pqa = psp.tile([CA, 512], dt.float32, tag="mm")
                pqb = psp.tile([CB, 512], dt.float32, tag="mm")
                for p, m0, mw in ((pqa, 0, CA), (pqb, CA, CB)):
                    for t, (ky, kx) in enumerate(taps):
                        rhs_a = q1a[:, r0 + ky:r0 + ky + 4, kx:kx + W]
                        rhs_b = q1b[:, r0 + ky:r0 + ky + 4, kx:kx + W]
                        nc.tensor.matmul(p[:], qdw_a[:, t, m0:m0 + mw], rhs_a,
                                         start=(t == 0), stop=False)
                        nc.tensor.matmul(p[:], qdw_b[:, t, m0:m0 + mw], rhs_b,
                                         start=False, stop=(t == 8))
                qe_a = tpp.tile([CA, 512], dt.bfloat16, tag="qe_a")
                qe_b = tpp.tile([CB, 512], dt.bfloat16, tag="qe_b")
                nc.any.tensor_copy(qe_a[:], pqa[:])
                nc.any.tensor_copy(qe_b[:], pqb[:])
                nc.sync.dma_start_transpose(qt_full[:, 4 * i:4 * (i + 1), 0:CA], qe_a[:])
                nc.scalar.dma_start_transpose(qt_full[:, 4 * i:4 * (i + 1), CA:C], qe_b[:])
                sq = evp.tile([CA, 512], dt.bfloat16, tag="sqscr")
                nc.scalar.activation(sq[:], qe_a[:], Act.Square,
                                     accum_out=ssq_a[:, i:i + 1])
                nc.scalar.activation(sq[0:CB], qe_b[:], Act.Square,
                                     accum_out=ssq_b[:, i:i + 1])

            # ---------- phase 4b: QK^T with just-in-time k transposes ----------
            for i in range(NCH):
                s0 = 512 * i
                kt4 = tpp.tile([128, 4, C], dt.bfloat16, tag="kt")
                nc.scalar.dma_start_transpose(kt4[:, :, 0:CA], ka[:, s0:s0 + 512])
                nc.sync.dma_start_transpose(kt4[:, :, CA:C], kvb_out[CB:CA, s0:s0 + 512])
                for j in range(4):
                    nc.tensor.matmul(attn_pa[:], qt_full[:, 4 * i + j, 0:CA], kt4[:, j, :],
                                     start=(i == 0 and j == 0),
                                     stop=(i == NCH - 1 and j == 3),
                                     skip_group_check=True)
                    nc.tensor.matmul(attn_pb[:], qt_full[:, 4 * i + j, CA:C], kt4[:, j, :],
                                     start=(i == 0 and j == 0),
                                     stop=(i == NCH - 1 and j == 3),
                                     skip_group_check=True)

            # ---------- phase 5: sum of squares of k rows ----------
            ssk_a = smp.tile([CA, 8], dt.float32)
            ssk_u = smp.tile([CA, 8], dt.float32)   # rows 64:128 active
            for (srct, plo, phi, dst) in ((ka, 0, CA, ssk_a), (kvb_out, CB, CA, ssk_u)):
                for i in range(8):
                    sq = evp.tile([CA, 1024], dt.bfloat16, tag="sqscr2")
                    nc.scalar.activation(sq[plo:phi], srct[plo:phi, 1024 * i:1024 * (i + 1)],
                                         Act.Square, accum_out=dst[plo:phi, i:i + 1])
            ssq = smp.tile([CA, 2], dt.float32)
            ssk = smp.tile([CA, 2], dt.float32)
            nc.vector.tensor_reduce(ssq[:, 0:1], ssq_a[:], mybir.AxisListType.X, Alu.add)
            nc.vector.tensor_reduce(ssq[0:CB, 1:2], ssq_b[:], mybir.AxisListType.X, Alu.add)
            nc.vector.tensor_reduce(ssk[:, 0:1], ssk_a[:], mybir.AxisListType.X, Alu.add)
            nc.vector.tensor_reduce(ssk[CB:CA, 1:2], ssk_u[CB:CA], mybir.AxisListType.X, Alu.add)

            # ---------- phase 6: pack + pairwise all-reduce ----------
            attn_sa = smp.tile([CA, C], dt.float32)
            attn_sb = smp.tile([CB, C], dt.float32)
            nc.any.tensor_copy(attn_sa[:], attn_pa[:])
            nc.any.tensor_copy(attn_sb[:], attn_pb[:])
            cin = dramp.tile([34, C], dt.float32)
            cout = dramp.tile([34, C], dt.float32)
            for h in range(HEADS):
                src = attn_sa if h < 4 else attn_sb
                r = HC * (h % 4)
                nc.sync.dma_start(cin[0:HC, HC * h:HC * (h + 1)],
                                  src[r:r + HC, HC * h:HC * (h + 1)])
            nc.sync.dma_start(cin[32:33, 0:CA].rearrange("o c -> c o"), ssq[:, 0:1])
            nc.sync.dma_start(cin[32:33, CA:C].rearrange("o c -> c o"), ssq[0:CB, 1:2])
            nc.sync.dma_start(cin[33:34, 0:CA].rearrange("o c -> c o"), ssk[:, 0:1])
            nc.sync.dma_start(cin[33:34, CA:C].rearrange("o c -> c o"), ssk[CB:CA, 1:2])
            nc.gpsimd.collective_compute(
                "AllReduce", Alu.add,
                replica_groups=[[0, 1], [2, 3], [4, 5], [6, 7]],
                ins=[cin[:].opt()], outs=[cout[:].opt()])

            # ---------- phase 7: softmax ----------
            attn_f = smp.tile([HC, HEADS, HC], dt.float32)
            nc.sync.dma_start(attn_f[:], cout[0:HC, :].rearrange("p (h c) -> p h c", h=HEADS))
            fq = smp.tile([HC, HEADS], dt.float32)
            fk = smp.tile([1, C], dt.float32)
            nc.sync.dma_start(fq[:], cout[32:33, :].rearrange("o (h c) -> c (o h)", h=HEADS))
            nc.sync.dma_start(fk[:], cout[33:34, :])
            for f in (fq, fk):
                nc.scalar.sqrt(f[:], f[:])
                nc.vector.tensor_scalar_max(f[:], f[:], 1e-12)
                nc.vector.reciprocal(f[:], f[:])
            nc.vector.tensor_tensor(fq[:], fq[:], temp_s[:], Alu.mult)
            fk32 = smp.tile([HC, C], dt.float32)
            nc.gpsimd.partition_broadcast(fk32[:], fk[:])
            nc.vector.tensor_tensor(attn_f[:], attn_f[:],
                                    fq[:, :, None].to_broadcast((HC, HEADS, HC)), Alu.mult)
            nc.vector.tensor_tensor(attn_f[:], attn_f[:],
                                    fk32[:].rearrange("p (h c) -> p h c", h=HEADS), Alu.mult)
            ex = smp.tile([HC, HEADS, HC], dt.float32)
            nc.scalar.activation(ex[:], attn_f[:], Act.Exp)
            sm = smp.tile([HC, HEADS], dt.float32)
            nc.vector.tensor_reduce(sm[:], ex[:], mybir.AxisListType.X, Alu.add)
            nc.vector.reciprocal(sm[:], sm[:])
            nc.vector.tensor_tensor(ex[:], ex[:],
                                    sm[:, :, None].to_broadcast((HC, HEADS, HC)), Alu.mult)
            attn_bf = smp.tile([HC, HEADS, HC], dt.bfloat16)
            nc.vector.tensor_copy(attn_bf[:], ex[:])
            attn_T = smp.tile([HC, HEADS, HC], dt.bfloat16)
            nc.vector.transpose(attn_T[:].rearrange("p h c -> p (h c)"),
                                attn_bf[:].rearrange("p h c -> p (h c)"))
            bd_a = smp.tile([CA, CA], dt.bfloat16)
            bd_b = smp.tile([CA, CB], dt.bfloat16)   # K-padded: rows 64:128 zero
            nc.vector.memset(bd_a[:], 0.0)
            nc.vector.memset(bd_b[:], 0.0)
            for h in range(HEADS):
                if h < 4:
                    nc.sync.dma_start(bd_a[HC * h:HC * (h + 1), HC * h:HC * (h + 1)],
                                      attn_T[:, h, :])
                else:
                    j = h - 4
                    nc.sync.dma_start(bd_b[HC * j:HC * (j + 1), HC * j:HC * (j + 1)],
                                      attn_T[:, h, :])

            # ---------- phase 8: out_heads = attn @ v ; proj ; residual (fused) ----------
            for i in range(NCH):
                s0 = 512 * i
                pva = psp.tile([CA, 512], dt.float32, tag="mm")
                pvb = psp.tile([CB, 512], dt.float32, tag="mm")
                nc.tensor.matmul(pva[:], bd_a[:], va[:, s0:s0 + 512], start=True, stop=True)
                # rhs rows 64:128 hold dw'd k-upper; bd_b zero rows cancel them
                nc.tensor.matmul(pvb[:], bd_b[:], kvb_out[:, s0:s0 + 512],
                                 start=True, stop=True)
                oha = ohp.tile([CA, 512], dt.bfloat16, tag="oh_a")
                ohb = ohp.tile([CA, 512], dt.bfloat16, tag="oh_b")
                nc.gpsimd.memset(ohb[CB:CA, :], 0.0)
                nc.any.tensor_copy(oha[:], pva[:])
                nc.any.tensor_copy(ohb[0:CB, :], pvb[:])
                ppa = psp.tile([CA, 512], dt.float32, tag="mm")
                ppb = psp.tile([CB, 512], dt.float32, tag="mm")
                nc.tensor.matmul(ppa[:], projw_a[:, 0:CA], oha[:], start=True, stop=False)
                nc.tensor.matmul(ppa[:], projw_b[:, 0:CA], ohb[:], start=False, stop=True)
                nc.tensor.matmul(ppb[:], projw_a[:, CA:C], oha[:], start=True, stop=False)
                nc.tensor.matmul(ppb[:], projw_b[:, CA:C], ohb[:], start=False, stop=True)
                xca = iop2.tile([CA, 512], dt.float32, tag="xc_a")
                xcb = iop2.tile([CB, 512], dt.float32, tag="xc_b")
                nc.sync.dma_start(xca[:], x_ctr_t.ap()[0:CA, s0:s0 + 512])
                nc.sync.dma_start(xcb[:], x_ctr_t.ap()[CA:C, s0:s0 + 512])
                nc.vector.scalar_tensor_tensor(xca[:], ppa[:], -1.0, xca[:], Alu.mult, Alu.add)
                nc.vector.scalar_tensor_tensor(xcb[:], ppb[:], -1.0, xcb[:], Alu.mult, Alu.add)
                nc.sync.dma_start(out_t.ap()[0:CA, s0:s0 + 512], xca[:])
                nc.sync.dma_start(out_t.ap()[CA:C, s0:s0 + 512], xcb[:])

    nc.compile()
    return nc


def _host_prep(inputs):
    x = np.asarray(inputs["x"], dtype=np.float32)
    y = np.asarray(inputs["y"], dtype=np.float32)
    kv_w = np.asarray(inputs["kv_w"], dtype=np.float32)[:, :, 0, 0]
    kv_dw = np.asarray(inputs["kv_dw_w"], dtype=np.float32)[:, 0]
    q_w = np.asarray(inputs["q_w"], dtype=np.float32)[:, :, 0, 0]
    q_dw = np.asarray(inputs["q_dw_w"], dtype=np.float32)
    proj_w = np.asarray(inputs["proj_w"], dtype=np.float32)[:, :, 0, 0]
    temp = np.asarray(inputs["temperature"], dtype=np.float32)[:, 0, 0]

    def kpad(a):  # [192, M] -> [256, M] with zero rows
        return np.concatenate([a, np.zeros((CP - C, a.shape[1]), a.dtype)], 0)

    # kv output-channel permutation: [k 0:128 | v 128:192 ; k 128:192 | v 0:128]
    perm = np.concatenate([np.arange(0, 128), np.arange(320, 384),
                           np.arange(128, 192), np.arange(192, 320)])
    kv_wT = np.ascontiguousarray(kpad(kv_w[perm].T)).astype(bf16)
    q_wT = np.ascontiguousarray(kpad(q_w.T)).astype(bf16)
    qdw_T = np.ascontiguousarray(
        np.stack([kpad(q_dw[:, :, ky, kx].T) for ky in range(3) for kx in range(3)])
    ).astype(bf16)
    kdw = kv_dw[:C].reshape(C, 9)
    vdw = kv_dw[C:].reshape(C, 9)
    dw_all = np.ascontiguousarray(np.concatenate(
        [kdw[0:128], vdw[128:192], kdw[128:192], vdw[0:128]], 0))
    proj_wT = np.ascontiguousarray(kpad(proj_w.T)).astype(bf16)
    temp2 = np.ascontiguousarray(np.broadcast_to(temp.reshape(1, HEADS), (HC, HEADS)))

    def shard(arr, b, s):
        r0 = HP * s
        p = np.zeros((CP, PH, PW), np.float32)
        lo, hi = max(r0 - 1, 0), min(r0 + HP + 1, H)
        p[:C, lo - r0 + 1:hi - r0 + 1, 1:W + 1] = arr[b, :, lo:hi, :]
        return np.ascontiguousarray(p.astype(bf16))

    in_maps = []
    for core in range(NCORES):
        b, s = core // 2, core % 2
        r0 = HP * s
        in_maps.append({
            "x_pad": shard(x, b, s),
            "y_pad": shard(y, b, s),
            "x_ctr": np.ascontiguousarray(
                x[b, :, r0:r0 + HP, :].reshape(C, S_IN)),
            "kv_wT": kv_wT, "q_wT": q_wT, "qdw_T": qdw_T,
            "dw_all": dw_all, "proj_wT": proj_wT,
            "temp": temp2,
        })
    return in_maps


LAST_RESULT = None


def kernel(**inputs):
    global LAST_RESULT
    from concourse.bass_utils import run_bass_kernel_spmd

    if "nc" not in _cache:
        _cache["nc"] = _build()
    nc = _cache["nc"]
    in_maps = _host_prep(inputs)
    res = run_bass_kernel_spmd(nc, in_maps, core_ids=list(range(NCORES)))
    LAST_RESULT = res
    out = np.empty((B, C, H, W), np.float32)
    for core in range(NCORES):
        b, s = core // 2, core % 2
        out[b, :, HP * s:HP * (s + 1), :] = \
            res.results[core]["out"].reshape(C, HP, W)
    return out
